# revision 15
# baseline (speedup 1.0000x reference)
import os
import sys

for _p in ("/opt/trn_rl_repo", "/root/.axon_site/_ro/trn_rl_repo"):
    if os.path.isdir(_p) and _p not in sys.path:
        sys.path.insert(0, _p)

import numpy as np
from concourse import bacc, tile, mybir
from concourse.bass_utils import run_bass_kernel_spmd

# Problem shapes (hardcoded per spec): x [32,1024,1024], W [3072,1024],
# bias [3072], A0/A1 [5,1024], B0/B1 [1024,5], s0/s1 scalar.
# out [32,1024,3072] = x @ (W + pad(cat(s0*B0@A0, s1*B1@A1)))^T + bias
# Sharding: data-parallel over batch, 4 batches (4096 tokens) per core.
#
# Per-core structure (v7):
#   - x and W are cast-loaded f32->bf16 by gpsimd (SWDGE) DMAs, then
#     transposed SBUF->SBUF by the XBAR (dma_start_transpose) into
#     chunk-major [128, nblk, 128] layout. Zero PE/DVE time for transposes.
#   - W'^T lives in 6 resident bf16 tiles Wb[ocb] = [128, 4(j), 8(d), 128(o)];
#     the rank-5 LoRA delta (replicated A / (s*B)^T factors) is matmul'd into
#     fp32 PSUM and DVE-added into the bf16 tiles for the K/V blocks.
#   - Main matmuls run bf16 x bf16 -> fp32 PSUM at 1 cycle/row (peak).
#   - DVE drains PSUM with the broadcast bias add; sync queue stores out.
#   - Host-side marshalling: (s*B).T precomputed on host (tiny [1024,5]
#     tensors; avoids a pathological strided gather on device).
B, S, D = 32, 1024, 1024
O = 3 * D
R = 5
N_CORES = 8
TOK = B * S // N_CORES          # 4096 tokens per core
P = 128
NO = 512                        # output free-dim chunk (one PSUM bank, fp32)
N_D = D // P                    # 8 contraction chunks
N_OC = O // NO                  # 6 output 512-blocks
N_SUP = TOK // NO               # 8 super chunks of 512 tokens
TC = NO // P                    # 4 token tiles per super chunk

F32 = mybir.dt.float32
F32R = mybir.dt.float32r
BF16 = mybir.dt.bfloat16

_CACHE = {}


def _build():
    nc = bacc.Bacc("TRN2", target_bir_lowering=False, debug=False,
                   num_devices=N_CORES)
    x_d = nc.declare_dram_parameter("x", [TOK, D], F32, isOutput=False)
    w_d = nc.declare_dram_parameter("w", [O, D], F32, isOutput=False)
    bias_d = nc.declare_dram_parameter("bias", [1, O], F32, isOutput=False)
    a0_d = nc.declare_dram_parameter("a0", [R, D], F32, isOutput=False)
    a1_d = nc.declare_dram_parameter("a1", [R, D], F32, isOutput=False)
    bt0_d = nc.declare_dram_parameter("bt0", [R, D], F32, isOutput=False)
    bt1_d = nc.declare_dram_parameter("bt1", [R, D], F32, isOutput=False)
    ones_d = nc.declare_dram_parameter("ones", [1, P], F32, isOutput=False)
    out_d = nc.declare_dram_parameter("out", [TOK, O], F32, isOutput=True)

    ADD = mybir.AluOpType.add

    with tile.TileContext(nc) as tc:
        with tc.tile_pool(name="const", bufs=1) as cpool, \
             tc.tile_pool(name="wres", bufs=1) as wpool, \
             tc.tile_pool(name="xload", bufs=8) as xbpool, \
             tc.tile_pool(name="wload", bufs=8) as wbpool, \
             tc.tile_pool(name="xt", bufs=2) as xtpool, \
             tc.tile_pool(name="ostage", bufs=4) as opool, \
             tc.tile_pool(name="psA", bufs=4, space="PSUM") as psA, \
             tc.tile_pool(name="psT", bufs=2, space="PSUM") as psT:

            # ---- small consts (sync queue) ----
            ones_sb = cpool.tile([1, P], F32R, tag="ones")
            nc.sync.dma_start(out=ones_sb[:], in_=ones_d[:].bitcast(F32R))
            bias1_sb = cpool.tile([1, O], F32R, tag="bias1")
            nc.sync.dma_start(out=bias1_sb[:], in_=bias_d[:].bitcast(F32R))
            a_sb = []
            for i, ad in enumerate((a0_d, a1_d)):
                t = cpool.tile([R, D], F32R, tag=f"a{i}", name=f"a{i}")
                nc.sync.dma_start(out=t[:], in_=ad[:].bitcast(F32R))
                a_sb.append(t)
            bt_sb = []
            for i, bd in enumerate((bt0_d, bt1_d)):
                t = cpool.tile([R, D], F32R, tag=f"bt{i}", name=f"bt{i}")
                nc.sync.dma_start(out=t[:], in_=bd[:].bitcast(F32R))
                bt_sb.append(t)

            # ---- resident W'^T: Wb[ocb] = [128, j, d, o] bf16 ----
            wt = [wpool.tile([P, TC, N_D, P], BF16, tag=f"wb{ocb}",
                             name=f"wb{ocb}") for ocb in range(N_OC)]

            # ---- x pipeline: gpsimd cast-load + XBAR transpose ----
            def emit_x_loads(sp):
                xb = []
                for tci in range(TC):
                    row0 = sp * NO + tci * P
                    t = xbpool.tile([P, D], BF16, tag="xb", name=f"xb{sp}_{tci}")
                    nc.gpsimd.dma_start(out=t[:], in_=x_d[row0:row0 + P, :])
                    xb.append(t)
                return xb

            def emit_x_transposes(sp, xb):
                xg = []
                for tci in range(TC):
                    t = xtpool.tile([P, N_D, P], BF16, tag=f"xg{tci}",
                                    name=f"xg{sp}_{tci}")
                    nc.scalar.dma_start_transpose(out=t[:], in_=xb[tci][:])
                    xg.append(t)
                return xg

            def emit_w_loads(ocb):
                wb = []
                for j in range(TC):
                    oc = ocb * TC + j
                    t = wbpool.tile([P, D], BF16, tag="wb", name=f"wl{oc}")
                    nc.gpsimd.dma_start(out=t[:],
                                        in_=w_d[oc * P:(oc + 1) * P, :])
                    wb.append(t)
                return wb

            def emit_w_transposes(ocb, wb):
                for j in range(TC):
                    nc.scalar.dma_start_transpose(out=wt[ocb][:, j, :, :],
                                                  in_=wb[j][:])

            # interleave startup DMA queues: x sp0, W ocb0, x sp1, W rest
            xb0 = emit_x_loads(0)
            wb_pend = {0: emit_w_loads(0)}
            xb1 = emit_x_loads(1)
            for ocb in range(1, N_OC):
                wb_pend[ocb] = emit_w_loads(ocb)

            xg_pending = {0: emit_x_transposes(0, xb0)}
            emit_w_transposes(0, wb_pend.pop(0))
            xg_pending[1] = emit_x_transposes(1, xb1)
            for ocb in range(1, N_OC):
                emit_w_transposes(ocb, wb_pend.pop(ocb))

            # ---- bias broadcast across partitions: [128, 3072] ----
            bias_bc = cpool.tile([P, O], F32, tag="biasbc")
            for j in range(N_OC):
                sl = slice(j * NO, (j + 1) * NO)
                b_ps = psA.tile([P, NO], F32, tag="acc")
                nc.tensor.matmul(b_ps[:], ones_sb[:], bias1_sb[:, sl],
                                 start=True, stop=True)
                nc.vector.tensor_copy(bias_bc[:, sl], b_ps[:])

            # ---- LoRA delta add for K/V blocks (ocb 2..5) ----
            def emit_lora(ocb, d):
                f = 0 if ocb < 4 else 1
                lo = ocb * NO - D - (D if f else 0)
                dl = psT.tile([P, NO], F32, tag="tp")
                nc.tensor.matmul(dl[:], a_sb[f][:, d * P:(d + 1) * P],
                                 bt_sb[f][:, lo:lo + NO], start=True, stop=True)
                sl = wt[ocb][:, :, d, :]
                nc.vector.tensor_tensor(out=sl, in0=dl[:], in1=sl, op=ADD)

            # ---- one accumulation group of main matmuls + drain + store ----
            def emit_acc_group(sp, tci, oc, xg):
                trow = slice(sp * NO + tci * P, sp * NO + (tci + 1) * P)
                osl = slice(oc * NO, (oc + 1) * NO)
                acc = psA.tile([P, NO], F32, tag="acc", name="acc")
                for d in range(N_D):
                    nc.tensor.matmul(acc[:], xg[tci][:, d, :],
                                     wt[oc][:, :, d, :],
                                     start=(d == 0), stop=(d == N_D - 1))
                o_sb = opool.tile([P, NO], F32, tag="ost", name="ost")
                nc.vector.tensor_tensor(out=o_sb[:], in0=acc[:],
                                        in1=bias_bc[:, osl], op=ADD)
                nc.sync.dma_start(out=out_d[trow, osl], in_=o_sb[:])

            # ---- startup: sp0/sp1 matmuls per ocb, LoRA interleaved ----
            for ocb in range(N_OC):
                if ocb >= 2:
                    for d in range(N_D):
                        emit_lora(ocb, d)
                for sp in (0, 1):
                    for tci in range(TC):
                        emit_acc_group(sp, tci, ocb, xg_pending[sp])

            # ---- steady state: superchunks 2..7 ----
            for sp in range(2, N_SUP):
                xb = emit_x_loads(sp)
                xg = emit_x_transposes(sp, xb)
                for tci in range(TC):
                    for oc in range(N_OC):
                        emit_acc_group(sp, tci, oc, xg)

    nc.compile()
    return nc


def kernel(x, W, bias, A0, A1, B0, B1, s0, s1, **run_kwargs):
    x = np.asarray(x, dtype=np.float32)
    if "nc" not in _CACHE:
        _CACHE["nc"] = _build()
    nc = _CACHE["nc"]

    s0 = np.float32(np.asarray(s0).reshape(()))
    s1 = np.float32(np.asarray(s1).reshape(()))
    shared = {
        "w": np.ascontiguousarray(np.asarray(W, np.float32)),
        "bias": np.asarray(bias, np.float32).reshape(1, O),
        "a0": np.ascontiguousarray(np.asarray(A0, np.float32)),
        "a1": np.ascontiguousarray(np.asarray(A1, np.float32)),
        "bt0": np.ascontiguousarray((s0 * np.asarray(B0, np.float32)).T),
        "bt1": np.ascontiguousarray((s1 * np.asarray(B1, np.float32)).T),
        "ones": np.ones((1, P), np.float32),
    }
    xr = x.reshape(N_CORES, TOK, D)
    in_maps = [{**shared, "x": np.ascontiguousarray(xr[c])} for c in range(N_CORES)]
    res = run_bass_kernel_spmd(nc, in_maps, list(range(N_CORES)), **run_kwargs)
    out = np.concatenate([res.results[c]["out"][None] for c in range(N_CORES)], 0)
    full = out.reshape(B, S, O)
    _CACHE["last_result"] = res
    return full
